# revision 39
# baseline (speedup 1.0000x reference)
"""GQA attention block (RMSNorm + QKV proj + partial RoPE + causal GQA
attention + XSA correction + out proj) on 8 trn2 NeuronCores.

Sharding: 2 batches x 4 KV-groups (each core: 1 batch, 1 kv head, 4 q heads).
Each core computes a partial output (its 4 heads through its wo column slice);
the host sums the 4 partials per batch.

v2 design (vs the fp32r baseline):
- all matmul operands bf16 (FWL weight loads, fast LDWEIGHTS, no gpsimd casts)
- causal mask folded into the score matmul as an identity @ mask-constant
  accumulation (exp of -1e6 underflows to 0) -- no gpsimd on the exp->PV path
- softmax denominator accumulated on the Vector engine (sumP += pT), one
  [1,TC] ones-matmul per (head,chunk) instead of one per key-tile
- rms scale rs computed from a row-major copy of x via tensor_tensor_reduce
  (no PE cycles, no x^2 elementwise muls)
- XSA + normalization via gpsimd partition_broadcast + DVE fast reciprocal
- software-pipelined issue order: next chunk's QKV projection and previous
  chunk's output projection matmuls are interleaved as "filler" into the
  scalar-bound attention phase so the PE never idles (keeps HAM at K=8/8)
"""

import sys
from collections import deque

for _p in ("/opt/trn_rl_repo", "/root/.axon_site/_ro/trn_rl_repo"):
    if _p not in sys.path:
        sys.path.append(_p)

import numpy as np
import ml_dtypes

import concourse.bass as bass
import concourse.bacc as bacc
import concourse.mybir as mybir
import concourse.tile as tile
from concourse.bass_utils import run_bass_kernel_spmd
from concourse.masks import make_identity

F32 = mybir.dt.float32
F32R = mybir.dt.float32r
BF16 = mybir.dt.bfloat16

B, T, D = 2, 2048, 2048
NH, NKV, HD = 16, 4, 128
RD = 64                    # rope dims
NH_L = NH // NKV           # 4 q heads per core
EL = (NH_L + 2) * HD       # 768: q0..q3, k, v
TC = 512                   # token chunk
NTC = T // TC              # 4
DC = D // 128              # 16 contraction chunks
S128 = float(1.0 / np.sqrt(HD))
EPS = 1e-6
MASKV = -1.0e6

_CACHE = {}


def _build_nc():
    nc = bacc.Bacc("TRN2", target_bir_lowering=False, debug=False)

    xTt = nc.declare_dram_parameter("xTt", [128, DC, T], BF16, isOutput=False)
    xR = nc.declare_dram_parameter("xR", [T, D], BF16, isOutput=False)
    wT = nc.declare_dram_parameter("wqkvT", [D, EL], BF16, isOutput=False)
    woL = nc.declare_dram_parameter("woL", [128, NH_L * D], BF16, isOutput=False)
    csP = nc.declare_dram_parameter("cs", [128, T], F32, isOutput=False)
    outp = nc.declare_dram_parameter("out", [T, D], F32, isOutput=True)

    ACT = mybir.ActivationFunctionType
    ALU = mybir.AluOpType

    with tile.TileContext(nc) as tc:
        with (
            nc.allow_low_precision(reason="bf16 matmuls; tolerance 2e-2"),
            tc.tile_pool(name="singles", bufs=1) as sg,
            tc.tile_pool(name="stream", bufs=2) as st,
            tc.tile_pool(name="ps", bufs=1, space="PSUM") as ps,
        ):
            # ---------------- persistent tiles ----------------
            w_sb = sg.tile([128, DC * EL], BF16, tag="w")
            wo_sb = sg.tile([128, NH_L * D], BF16, tag="wo")
            cos_sb = sg.tile([RD, T], F32, tag="cos")
            sinS_sb = sg.tile([RD, T], F32, tag="sin")
            identf = sg.tile([128, 128], F32, tag="identf")
            identb = sg.tile([128, 128], BF16, tag="identb")
            ones_c = sg.tile([128, 1], F32R, tag="ones_c")
            ones_f = sg.tile([128, 1], F32, tag="ones_f")
            ones_cb = sg.tile([128, 1], BF16, tag="ones_cb")
            ones_r = sg.tile([1, 128], F32R, tag="ones_r")
            ones_rf = sg.tile([1, 128], F32, tag="ones_rf")
            eps_t = sg.tile([128, 1], F32, tag="eps_t")
            qhat = [[sg.tile([128, TC], BF16, tag=f"qh{h}_{r}", name=f"qh{h}_{r}")
                     for r in range(2)] for h in range(NH_L)]
            khat = [sg.tile([128, TC], BF16, tag=f"kh{j}", name=f"kh{j}")
                    for j in range(NTC)]
            vhat = [sg.tile([128, TC], BF16, tag=f"vh{j}", name=f"vh{j}")
                    for j in range(NTC)]
            vtok = [sg.tile([128, TC], BF16, tag=f"vt{j}", name=f"vt{j}")
                    for j in range(NTC)]
            rvnsb = [sg.tile([128, TC], F32, tag=f"rvns{j}", name=f"rvns{j}")
                     for j in range(NTC)]
            aout = [[sg.tile([128, TC], BF16, tag=f"ao{h}_{r}", name=f"ao{h}_{r}")
                     for r in range(2)] for h in range(NH_L)]
            rsb = [sg.tile([128, TC], F32, tag=f"rsb{r}", name=f"rsb{r}")
                   for r in range(2)]

            # ---------------- init ----------------
            nc.sync.dma_start(out=cos_sb, in_=csP[0:RD, :])
            nc.sync.dma_start(out=sinS_sb, in_=csP[RD:128, :])
            for h in range(NH_L):
                nc.sync.dma_start(out=wo_sb[:, h * D:(h + 1) * D],
                                  in_=woL[:, h * D:(h + 1) * D])
            for i in range(DC):
                nc.sync.dma_start(out=w_sb[:, i * EL:(i + 1) * EL],
                                  in_=wT[i * 128:(i + 1) * 128, :])
            make_identity(nc, identf)
            nc.gpsimd.tensor_copy(identb, identf)
            nc.vector.memset(ones_f, 1.0)
            nc.scalar.copy(ones_c, ones_f)
            nc.vector.memset(ones_cb, 1.0)
            nc.vector.memset(ones_rf, 1.0)
            nc.scalar.copy(ones_r, ones_rf)
            nc.vector.memset(eps_t, EPS)

            # ---------------- filler machinery ----------------
            fill_q = deque()

            def emit_fill(n):
                done = 0
                while fill_q and done < n:
                    try:
                        next(fill_q[0])
                        done += 1
                    except StopIteration:
                        fill_q.popleft()

            def drain_fill():
                while fill_q:
                    try:
                        next(fill_q[0])
                    except StopIteration:
                        fill_q.popleft()

            # ---------------- chunk prep (QKV + rs + rope + vtok + vns) ----
            def prep_gen(jn):
                js = slice(jn * TC, (jn + 1) * TC)
                r = jn % 2

                # x row-major tiles -> per-token sum of squares -> rs row
                srow_sb = st.tile([1, TC], F32R, tag="row", bufs=4, name="srow_sb")
                for kk in range(4):
                    xr = st.tile([128, D], BF16, tag="xr", bufs=4,
                                 name=f"xr{jn}_{kk}")
                    nc.sync.dma_start(
                        out=xr, in_=xR[jn * TC + kk * 128: jn * TC + (kk + 1) * 128, :])
                    xsq = st.tile([128, D], BF16, tag="xsq", bufs=1)
                    nc.vector.tensor_mul(xsq, xr, xr)
                    ssq = st.tile([128, 1], F32R, tag="ssq", bufs=8)
                    nc.vector.tensor_reduce(
                        ssq, xsq, axis=mybir.AxisListType.X, op=ALU.add)
                    nc.sync.dma_start(
                        out=srow_sb[0:1, kk * 128:(kk + 1) * 128], in_=ssq)
                    yield
                ms_b = ps.tile([128, TC], F32, tag="F", bufs=2, name="ms_b")
                nc.tensor.matmul(ms_b, ones_r, srow_sb, start=True, stop=True)
                sq_b = st.tile([128, TC], F32, tag="bc", bufs=8, name="sq_b")
                nc.scalar.activation(sq_b, ms_b, ACT.Sqrt, scale=1.0 / D,
                                     bias=eps_t)
                nc.vector.reciprocal_approx_fast(rsb[r], sq_b)
                yield

                # all 16 contraction tiles for this chunk in ONE DMA/tile
                xt_all = st.tile([128, DC * TC], BF16, tag="xt", bufs=2,
                                 name=f"xt{jn}")
                nc.sync.dma_start(out=xt_all, in_=xTt[:, :, js])
                yield

                # QKV projection, output-major (k and v first so the shared
                # tensors and their derived chains finish earliest)
                for e in (NH_L, NH_L + 1, 0, 1, 2, 3):
                    qk = ps.tile([128, TC], F32, tag="F", bufs=2,
                                 name=f"qk{jn}_{e}")
                    for i in range(DC):
                        nc.tensor.matmul(
                            qk,
                            w_sb[:, i * EL + e * 128: i * EL + (e + 1) * 128],
                            xt_all[:, i * TC:(i + 1) * TC],
                            start=(i == 0), stop=(i == DC - 1),
                        )
                        if i % 2 == 1:
                            yield
                    if e < NH_L:
                        dest = qhat[e][r]
                    elif e == NH_L:
                        dest = khat[jn]
                    else:
                        dest = vhat[jn]
                    nc.vector.tensor_mul(dest, qk, rsb[r])
                    yield
                    # rope for q heads and k (not v)
                    if e <= NH_L:
                        t2 = st.tile([RD, TC], BF16, tag="t2", bufs=2)
                        nc.sync.dma_start(out=t2[0:32], in_=dest[32:64])
                        nc.sync.dma_start(out=t2[32:64], in_=dest[0:32])
                        nc.vector.tensor_mul(t2, t2, sinS_sb[:, js])
                        t1 = st.tile([RD, TC], BF16, tag="t1", bufs=2)
                        nc.vector.tensor_mul(t1, dest[0:RD], cos_sb[:, js])
                        nc.vector.tensor_add(dest[0:RD], t1, t2)
                        yield
                    if e == NH_L + 1:
                        # vtok = vhat^T (token-major v) via identity matmuls
                        vtp = ps.tile([128, TC], F32, tag="F", bufs=2,
                                      name=f"vtp{jn}")
                        for kk in range(4):
                            nc.tensor.matmul(
                                vtp[:, kk * 128:(kk + 1) * 128],
                                vhat[jn][:, kk * 128:(kk + 1) * 128],
                                identb,
                                start=True, stop=True)
                            yield
                        nc.scalar.copy(vtok[jn], vtp)
                        yield
                        # rvns = 1 / (sum_hd v^2 + eps) from token-major vtok
                        vrow_sb = st.tile([1, TC], F32R, tag="row", bufs=4,
                                          name="vrow_sb")
                        for kk in range(4):
                            vsqk = st.tile([128, 128], BF16, tag="vsqk", bufs=1)
                            nc.vector.tensor_mul(
                                vsqk, vtok[jn][:, kk * 128:(kk + 1) * 128],
                                vtok[jn][:, kk * 128:(kk + 1) * 128])
                            vssq = st.tile([128, 1], F32R, tag="ssq", bufs=8)
                            nc.vector.tensor_reduce(
                                vssq, vsqk, axis=mybir.AxisListType.X, op=ALU.add)
                            nc.sync.dma_start(
                                out=vrow_sb[0:1, kk * 128:(kk + 1) * 128],
                                in_=vssq)
                        yield
                        vb = ps.tile([128, TC], F32, tag="F", bufs=2, name="vb")
                        nc.tensor.matmul(vb, ones_r, vrow_sb,
                                         start=True, stop=True)
                        vb2 = st.tile([128, TC], F32, tag="bc", bufs=8,
                                      name="vb2")
                        nc.vector.tensor_scalar_add(vb2, vb, EPS)
                        nc.vector.reciprocal_approx_fast(rvnsb[jn], vb2)
                        yield

            # ---------------- output projection for chunk jo --------------
            def outproj_gen(jo):
                r = jo % 2
                for tt in range(4):
                    for m in range(4):
                        po = ps.tile([128, TC], F32, tag="F", bufs=2,
                                     name=f"po{jo}_{tt}_{m}")
                        for h in range(NH_L):
                            nc.tensor.matmul(
                                po,
                                aout[h][r][:, tt * 128:(tt + 1) * 128],
                                wo_sb[:, h * D + m * TC: h * D + (m + 1) * TC],
                                start=(h == 0), stop=(h == NH_L - 1),
                            )
                            if h % 2 == 1:
                                yield
                        osb = st.tile([128, TC], F32, tag="osb", bufs=4)
                        nc.vector.tensor_copy(osb, po)
                        nc.sync.dma_start(
                            out=outp[jo * TC + tt * 128: jo * TC + (tt + 1) * 128,
                                     m * TC:(m + 1) * TC],
                            in_=osb)
                        yield

            # ---------------- attention for chunk j ----------------
            def attention(j):
                r = j % 2
                nkt = 4 * (j + 1)
                for h in range(NH_L):
                    pvh = ps.tile([128, TC], F32, tag="PV", bufs=2,
                                  name=f"pv{j}_{h}")
                    drow = ps.tile([1, TC], F32, tag="S2", bufs=2,
                                   name=f"den{j}_{h}")
                    for kt in range(nkt):
                        jk = kt // 4
                        ksl = slice((kt % 4) * 128, (kt % 4 + 1) * 128)
                        diag = (kt >= 4 * j)
                        sc = ps.tile([128, TC], F32, tag="SC", bufs=2)
                        nc.tensor.matmul(sc, khat[jk][:, ksl], qhat[h][r],
                                         start=True, stop=True)
                        pt = st.tile([128, TC], BF16, tag="pT", bufs=4)
                        nc.scalar.activation(pt, sc, ACT.Exp, scale=S128)
                        if diag:
                            m = kt - 4 * j
                            nc.gpsimd.affine_select(
                                out=pt, in_=pt,
                                compare_op=ALU.is_ge, fill=0.0,
                                base=-m * 128, pattern=[[1, TC]],
                                channel_multiplier=-1,
                            )
                        emit_fill(3)
                        nc.tensor.matmul(drow, ones_cb, pt,
                                         start=(kt == 0), stop=(kt == nkt - 1))
                        nc.tensor.matmul(pvh, vtok[jk][:, ksl], pt,
                                         start=(kt == 0), stop=(kt == nkt - 1))

                    # head epilogue: normalization + XSA correction.
                    # broadcasts are PE rank-1 matmuls (no gpsimd hops).
                    den_sb = st.tile([1, TC], F32R, tag="row", bufs=4,
                                     name="den_sb")
                    nc.scalar.copy(den_sb, drow)
                    den_b = ps.tile([128, TC], F32, tag="SC", bufs=2,
                                    name="den_b")
                    nc.tensor.matmul(den_b, ones_r, den_sb, start=True, stop=True)
                    inv_b = st.tile([128, TC], F32, tag="bc", bufs=8, name="inv_b")
                    nc.vector.reciprocal_approx_fast(inv_b, den_b)
                    pvs = st.tile([128, TC], F32, tag="pv", bufs=2, name="pvs")
                    nc.scalar.copy(pvs, pvh)
                    tu = st.tile([128, TC], F32R, tag="tu", bufs=2, name="tu")
                    nc.vector.tensor_mul(tu, pvs, vhat[j])
                    emit_fill(6)
                    dotrow = ps.tile([1, TC], F32, tag="S2", bufs=2,
                                     name=f"dot{j}_{h}")
                    nc.tensor.matmul(dotrow, ones_c, tu, start=True, stop=True)
                    dot_sb = st.tile([1, TC], F32R, tag="row", bufs=4,
                                     name="dot_sb")
                    nc.scalar.copy(dot_sb, dotrow)
                    dot_b = ps.tile([128, TC], F32, tag="SC", bufs=2,
                                    name="dot_b")
                    nc.tensor.matmul(dot_b, ones_r, dot_sb, start=True, stop=True)
                    f_b = st.tile([128, TC], F32, tag="bc", bufs=8, name="f_b")
                    nc.vector.tensor_mul(f_b, dot_b, rvnsb[j])
                    m2 = st.tile([128, TC], F32, tag="m2", bufs=2, name="m2")
                    nc.vector.tensor_mul(m2, vhat[j], f_b)
                    nc.vector.tensor_sub(m2, pvs, m2)
                    nc.vector.tensor_mul(aout[h][r], m2, inv_b)
                    emit_fill(3)

            # ---------------- schedule ----------------
            # chunk 0 prep runs solid (nothing to interleave with)
            for _ in prep_gen(0):
                pass
            for j in range(NTC):
                if j + 1 < NTC:
                    fill_q.append(prep_gen(j + 1))
                if j >= 1:
                    fill_q.append(outproj_gen(j - 1))
                attention(j)
                drain_fill()
            for _ in outproj_gen(NTC - 1):
                pass

    nc.compile()
    return nc


def _host_inputs(x, cos, sin, w_norm, wq, wk, wv, wo):
    """Build the 8 per-core input maps (host-side layout prep only)."""
    bf = ml_dtypes.bfloat16
    wn = w_norm.astype(np.float32)
    cosT = cos.T.astype(np.float32)                                # [64, T]
    sinT = sin.T.astype(np.float32)
    sinS = np.concatenate([-sinT[:32], sinT[32:]], axis=0)         # [64, T]
    cs = np.ascontiguousarray(
        np.concatenate([cosT, sinS], axis=0), dtype=np.float32)    # [128, T]
    xb = x.astype(bf)
    xTts = [np.ascontiguousarray(xb[b].reshape(T, DC, 128).transpose(2, 1, 0))
            for b in range(B)]
    xRs = [np.ascontiguousarray(xb[b]) for b in range(B)]
    in_maps = []
    for c in range(8):
        b, g = divmod(c, 4)
        wq_s = wq[g * NH_L * HD:(g + 1) * NH_L * HD] * wn[None, :]
        wk_s = wk[g * HD:(g + 1) * HD] * wn[None, :]
        wv_s = wv[g * HD:(g + 1) * HD] * wn[None, :]
        wqkvT = np.ascontiguousarray(
            np.concatenate([wq_s, wk_s, wv_s], axis=0).T).astype(bf)  # [D, 768]
        # woL[p, h*D + d] = wo[d, g*512 + h*128 + p]
        woL = np.ascontiguousarray(
            wo[:, g * NH_L * HD:(g + 1) * NH_L * HD]                 # [D, 512]
            .T.reshape(NH_L, HD, D).transpose(1, 0, 2).reshape(HD, NH_L * D)
        ).astype(bf)                                                 # [128, 4*D]
        in_maps.append({
            "xTt": xTts[b],
            "xR": xRs[b],
            "wqkvT": wqkvT,
            "woL": woL,
            "cs": cs,
        })
    return in_maps


def kernel(x, cos, sin, w_norm, wq, wk, wv, wo, rope_dims=64, use_xsa=1,
           **_unused):
    if "nc" not in _CACHE:
        _CACHE["nc"] = _build_nc()
    nc = _CACHE["nc"]
    in_maps = _host_inputs(
        np.asarray(x), np.asarray(cos), np.asarray(sin), np.asarray(w_norm),
        np.asarray(wq), np.asarray(wk), np.asarray(wv), np.asarray(wo),
    )
    res_obj = run_bass_kernel_spmd(nc, in_maps, list(range(8)))
    _CACHE["last"] = res_obj
    res = res_obj.results
    out = np.zeros((B, T, D), dtype=np.float32)
    for c in range(8):
        b = c // 4
        out[b] += np.asarray(res[c]["out"], dtype=np.float32)
    return out


# revision 44
# speedup vs baseline: 1.0708x; 1.0708x over previous
"""GQA attention block (RMSNorm + QKV proj + partial RoPE + causal GQA
attention + XSA correction + out proj) on 8 trn2 NeuronCores.

Sharding: 2 batches x 4 KV-groups (each core: 1 batch, 1 kv head, 4 q heads).
Each core computes a partial output (its 4 heads through its wo column slice);
the host sums the 4 partials per batch.

v2 design (vs the fp32r baseline):
- all matmul operands bf16 (FWL weight loads, fast LDWEIGHTS, no gpsimd casts)
- causal mask folded into the score matmul as an identity @ mask-constant
  accumulation (exp of -1e6 underflows to 0) -- no gpsimd on the exp->PV path
- softmax denominator accumulated on the Vector engine (sumP += pT), one
  [1,TC] ones-matmul per (head,chunk) instead of one per key-tile
- rms scale rs computed from a row-major copy of x via tensor_tensor_reduce
  (no PE cycles, no x^2 elementwise muls)
- XSA + normalization via gpsimd partition_broadcast + DVE fast reciprocal
- software-pipelined issue order: next chunk's QKV projection and previous
  chunk's output projection matmuls are interleaved as "filler" into the
  scalar-bound attention phase so the PE never idles (keeps HAM at K=8/8)
"""

import sys
from collections import deque

for _p in ("/opt/trn_rl_repo", "/root/.axon_site/_ro/trn_rl_repo"):
    if _p not in sys.path:
        sys.path.append(_p)

import numpy as np
import ml_dtypes

import concourse.bass as bass
import concourse.bacc as bacc
import concourse.mybir as mybir
import concourse.tile as tile
from concourse.bass_utils import run_bass_kernel_spmd
from concourse.masks import make_identity

F32 = mybir.dt.float32
F32R = mybir.dt.float32r
BF16 = mybir.dt.bfloat16

B, T, D = 2, 2048, 2048
NH, NKV, HD = 16, 4, 128
RD = 64                    # rope dims
NH_L = NH // NKV           # 4 q heads per core
EL = (NH_L + 2) * HD       # 768: q0..q3, k, v
TC = 512                   # token chunk
NTC = T // TC              # 4
DC = D // 128              # 16 contraction chunks
S128 = float(1.0 / np.sqrt(HD))
EPS = 1e-6
MASKV = -1.0e6

_CACHE = {}


def _build_nc():
    nc = bacc.Bacc("TRN2", target_bir_lowering=False, debug=False)

    xTt = nc.declare_dram_parameter("xTt", [128, DC, T], BF16, isOutput=False)
    xR = nc.declare_dram_parameter("xR", [T, D], BF16, isOutput=False)
    wT = nc.declare_dram_parameter("wqkvT", [D, EL], BF16, isOutput=False)
    woL = nc.declare_dram_parameter("woL", [128, NH_L * D], BF16, isOutput=False)
    csP = nc.declare_dram_parameter("cs", [128, T], F32, isOutput=False)
    outp = nc.declare_dram_parameter("out", [T, D], F32, isOutput=True)

    ACT = mybir.ActivationFunctionType
    ALU = mybir.AluOpType

    with tile.TileContext(nc) as tc:
        with (
            nc.allow_low_precision(reason="bf16 matmuls; tolerance 2e-2"),
            tc.tile_pool(name="singles", bufs=1) as sg,
            tc.tile_pool(name="stream", bufs=2) as st,
            tc.tile_pool(name="ps", bufs=1, space="PSUM") as ps,
        ):
            # ---------------- persistent tiles ----------------
            w_sb = sg.tile([128, DC * EL], BF16, tag="w")
            wo_sb = sg.tile([128, NH_L * D], BF16, tag="wo")
            cos_sb = sg.tile([RD, T], F32, tag="cos")
            sinS_sb = sg.tile([RD, T], F32, tag="sin")
            identf = sg.tile([128, 128], F32, tag="identf")
            identb = sg.tile([128, 128], BF16, tag="identb")
            ones_c = sg.tile([128, 1], F32R, tag="ones_c")
            ones_f = sg.tile([128, 1], F32, tag="ones_f")
            ones_cb = sg.tile([128, 1], BF16, tag="ones_cb")
            ones_r = sg.tile([1, 128], F32R, tag="ones_r")
            ones_rf = sg.tile([1, 128], F32, tag="ones_rf")
            eps_t = sg.tile([128, 1], F32, tag="eps_t")
            qhat = [[sg.tile([128, TC], BF16, tag=f"qh{h}_{r}", name=f"qh{h}_{r}")
                     for r in range(2)] for h in range(NH_L)]
            khat = [sg.tile([128, TC], BF16, tag=f"kh{j}", name=f"kh{j}")
                    for j in range(NTC)]
            vhat = [sg.tile([128, TC], BF16, tag=f"vh{j}", name=f"vh{j}")
                    for j in range(NTC)]
            vtok = [sg.tile([128, TC], BF16, tag=f"vt{j}", name=f"vt{j}")
                    for j in range(NTC)]
            rvnsb = [sg.tile([128, TC], F32, tag=f"rvns{j}", name=f"rvns{j}")
                     for j in range(NTC)]
            aout = [[sg.tile([128, TC], BF16, tag=f"ao{h}_{r}", name=f"ao{h}_{r}")
                     for r in range(2)] for h in range(NH_L)]
            rsb = [sg.tile([128, TC], F32, tag=f"rsb{r}", name=f"rsb{r}")
                   for r in range(2)]

            # ---------------- init ----------------
            for i in range(DC):
                nc.sync.dma_start(out=w_sb[:, i * EL:(i + 1) * EL],
                                  in_=wT[i * 128:(i + 1) * 128, :])
            nc.sync.dma_start(out=cos_sb, in_=csP[0:RD, :])
            nc.sync.dma_start(out=sinS_sb, in_=csP[RD:128, :])
            make_identity(nc, identf)
            nc.gpsimd.tensor_copy(identb, identf)
            nc.vector.memset(ones_f, 1.0)
            nc.scalar.copy(ones_c, ones_f)
            nc.vector.memset(ones_cb, 1.0)
            nc.vector.memset(ones_rf, 1.0)
            nc.scalar.copy(ones_r, ones_rf)
            nc.vector.memset(eps_t, EPS)

            # ---------------- filler machinery ----------------
            fill_q = deque()

            def emit_fill(n):
                done = 0
                while fill_q and done < n:
                    try:
                        next(fill_q[0])
                        done += 1
                    except StopIteration:
                        fill_q.popleft()

            def drain_fill():
                while fill_q:
                    try:
                        next(fill_q[0])
                    except StopIteration:
                        fill_q.popleft()

            # ---------------- chunk prep (QKV + rs + rope + vtok + vns) ----
            def prep_gen(jn):
                js = slice(jn * TC, (jn + 1) * TC)
                r = jn % 2

                # all 16 contraction tiles for this chunk in ONE DMA/tile
                xt_all = st.tile([128, DC * TC], BF16, tag="xt", bufs=2,
                                 name=f"xt{jn}")
                nc.sync.dma_start(out=xt_all, in_=xTt[:, :, js])

                # x row-major tiles -> per-token sum of squares -> rs row
                srow_sb = st.tile([1, TC], F32R, tag="row", bufs=4, name="srow_sb")
                for kk in range(4):
                    xr = st.tile([128, D], BF16, tag="xr", bufs=4,
                                 name=f"xr{jn}_{kk}")
                    nc.sync.dma_start(
                        out=xr, in_=xR[jn * TC + kk * 128: jn * TC + (kk + 1) * 128, :])
                    xsq = st.tile([128, D], BF16, tag="xsq", bufs=1)
                    nc.vector.tensor_mul(xsq, xr, xr)
                    ssq = st.tile([128, 1], F32R, tag="ssq", bufs=8)
                    nc.vector.tensor_reduce(
                        ssq, xsq, axis=mybir.AxisListType.X, op=ALU.add)
                    nc.sync.dma_start(
                        out=srow_sb[0:1, kk * 128:(kk + 1) * 128], in_=ssq)
                    yield
                ms_b = ps.tile([128, TC], F32, tag="F", bufs=2, name="ms_b")
                nc.tensor.matmul(ms_b, ones_r, srow_sb, start=True, stop=True)
                sq_b = st.tile([128, TC], F32, tag="bc", bufs=8, name="sq_b")
                nc.scalar.activation(sq_b, ms_b, ACT.Sqrt, scale=1.0 / D,
                                     bias=eps_t)
                nc.vector.reciprocal_approx_fast(rsb[r], sq_b)
                yield

                # QKV projection, output-major (k and v first so the shared
                # tensors and their derived chains finish earliest)
                for e in (NH_L, NH_L + 1, 0, 1, 2, 3):
                    qk = ps.tile([128, TC], F32, tag="F", bufs=2,
                                 name=f"qk{jn}_{e}")
                    for i in range(DC):
                        nc.tensor.matmul(
                            qk,
                            w_sb[:, i * EL + e * 128: i * EL + (e + 1) * 128],
                            xt_all[:, i * TC:(i + 1) * TC],
                            start=(i == 0), stop=(i == DC - 1),
                        )
                        if i % 2 == 1:
                            yield
                    if e < NH_L:
                        dest = qhat[e][r]
                    elif e == NH_L:
                        dest = khat[jn]
                    else:
                        dest = vhat[jn]
                    nc.vector.tensor_mul(dest, qk, rsb[r])
                    yield
                    # rope for q heads and k (not v)
                    if e <= NH_L:
                        t2 = st.tile([RD, TC], BF16, tag="t2", bufs=2)
                        nc.sync.dma_start(out=t2[0:32], in_=dest[32:64])
                        nc.sync.dma_start(out=t2[32:64], in_=dest[0:32])
                        nc.vector.tensor_mul(t2, t2, sinS_sb[:, js])
                        t1 = st.tile([RD, TC], BF16, tag="t1", bufs=2)
                        nc.vector.tensor_mul(t1, dest[0:RD], cos_sb[:, js])
                        nc.vector.tensor_add(dest[0:RD], t1, t2)
                        yield
                    if e == NH_L + 1:
                        # vtok = vhat^T (token-major v) via identity matmuls
                        vtp = ps.tile([128, TC], F32, tag="F", bufs=2,
                                      name=f"vtp{jn}")
                        for kk in range(4):
                            nc.tensor.matmul(
                                vtp[:, kk * 128:(kk + 1) * 128],
                                vhat[jn][:, kk * 128:(kk + 1) * 128],
                                identb,
                                start=True, stop=True)
                            yield
                        nc.scalar.copy(vtok[jn], vtp)
                        yield
                        # rvns = 1 / (sum_hd v^2 + eps) from token-major vtok
                        vrow_sb = st.tile([1, TC], F32R, tag="row", bufs=4,
                                          name="vrow_sb")
                        for kk in range(4):
                            vsqk = st.tile([128, 128], BF16, tag="vsqk", bufs=1)
                            nc.vector.tensor_mul(
                                vsqk, vtok[jn][:, kk * 128:(kk + 1) * 128],
                                vtok[jn][:, kk * 128:(kk + 1) * 128])
                            vssq = st.tile([128, 1], F32R, tag="ssq", bufs=8)
                            nc.vector.tensor_reduce(
                                vssq, vsqk, axis=mybir.AxisListType.X, op=ALU.add)
                            nc.sync.dma_start(
                                out=vrow_sb[0:1, kk * 128:(kk + 1) * 128],
                                in_=vssq)
                        yield
                        vb = ps.tile([128, TC], F32, tag="F", bufs=2, name="vb")
                        nc.tensor.matmul(vb, ones_r, vrow_sb,
                                         start=True, stop=True)
                        vb2 = st.tile([128, TC], F32, tag="bc", bufs=8,
                                      name="vb2")
                        nc.vector.tensor_scalar_add(vb2, vb, EPS)
                        nc.vector.reciprocal_approx_fast(rvnsb[jn], vb2)
                        yield

            # ---------------- output projection for chunk jo --------------
            def outproj_gen(jo):
                r = jo % 2
                for tt in range(4):
                    for m in range(4):
                        po = ps.tile([128, TC], F32, tag="F", bufs=2,
                                     name=f"po{jo}_{tt}_{m}")
                        for h in range(NH_L):
                            nc.tensor.matmul(
                                po,
                                aout[h][r][:, tt * 128:(tt + 1) * 128],
                                wo_sb[:, h * D + m * TC: h * D + (m + 1) * TC],
                                start=(h == 0), stop=(h == NH_L - 1),
                            )
                            if h % 2 == 1:
                                yield
                        osb = st.tile([128, TC], F32, tag="osb", bufs=4)
                        nc.vector.tensor_copy(osb, po)
                        nc.sync.dma_start(
                            out=outp[jo * TC + tt * 128: jo * TC + (tt + 1) * 128,
                                     m * TC:(m + 1) * TC],
                            in_=osb)
                        yield

            # ---------------- attention for chunk j ----------------
            def attention(j):
                r = j % 2
                nkt = 4 * (j + 1)
                for h in range(NH_L):
                    pvh = ps.tile([128, TC], F32, tag="PV", bufs=2,
                                  name=f"pv{j}_{h}")
                    drow = ps.tile([1, TC], F32, tag="S2", bufs=2,
                                   name=f"den{j}_{h}")
                    for kp in range(nkt // 2):
                        scp = ps.tile([128, 2 * TC], F32, tag="SC", bufs=1)
                        for half in range(2):
                            kt = 2 * kp + half
                            jk = kt // 4
                            ksl = slice((kt % 4) * 128, (kt % 4 + 1) * 128)
                            nc.tensor.matmul(
                                scp[:, half * TC:(half + 1) * TC],
                                khat[jk][:, ksl], qhat[h][r],
                                start=True, stop=True)
                        ptp = st.tile([128, 2 * TC], BF16, tag="pT", bufs=3)
                        nc.scalar.activation(ptp, scp, ACT.Exp, scale=S128)
                        for half in range(2):
                            kt = 2 * kp + half
                            if kt >= 4 * j:
                                m = kt - 4 * j
                                nc.gpsimd.affine_select(
                                    out=ptp[:, half * TC:(half + 1) * TC],
                                    in_=ptp[:, half * TC:(half + 1) * TC],
                                    compare_op=ALU.is_ge, fill=0.0,
                                    base=-m * 128, pattern=[[1, TC]],
                                    channel_multiplier=-1,
                                )
                        emit_fill(4)
                        for half in range(2):
                            kt = 2 * kp + half
                            jk = kt // 4
                            ksl = slice((kt % 4) * 128, (kt % 4 + 1) * 128)
                            pts = ptp[:, half * TC:(half + 1) * TC]
                            nc.tensor.matmul(drow, ones_cb, pts,
                                             start=(kt == 0),
                                             stop=(kt == nkt - 1))
                            nc.tensor.matmul(pvh, vtok[jk][:, ksl], pts,
                                             start=(kt == 0),
                                             stop=(kt == nkt - 1))

                    # head epilogue: normalization + XSA correction.
                    # broadcasts are PE rank-1 matmuls (no gpsimd hops).
                    den_sb = st.tile([1, TC], F32R, tag="row", bufs=4,
                                     name="den_sb")
                    nc.scalar.copy(den_sb, drow)
                    den_b = ps.tile([128, TC], F32, tag="F", bufs=2,
                                    name="den_b")
                    nc.tensor.matmul(den_b, ones_r, den_sb, start=True, stop=True)
                    inv_b = st.tile([128, TC], F32, tag="bc", bufs=8, name="inv_b")
                    nc.vector.reciprocal_approx_fast(inv_b, den_b)
                    pvs = st.tile([128, TC], F32, tag="pv", bufs=2, name="pvs")
                    nc.scalar.copy(pvs, pvh)
                    tu = st.tile([128, TC], F32R, tag="tu", bufs=2, name="tu")
                    nc.vector.tensor_mul(tu, pvs, vhat[j])
                    emit_fill(6)
                    dotrow = ps.tile([1, TC], F32, tag="S2", bufs=2,
                                     name=f"dot{j}_{h}")
                    nc.tensor.matmul(dotrow, ones_c, tu, start=True, stop=True)
                    dot_sb = st.tile([1, TC], F32R, tag="row", bufs=4,
                                     name="dot_sb")
                    nc.scalar.copy(dot_sb, dotrow)
                    dot_b = ps.tile([128, TC], F32, tag="F", bufs=2,
                                    name="dot_b")
                    nc.tensor.matmul(dot_b, ones_r, dot_sb, start=True, stop=True)
                    f_b = st.tile([128, TC], F32, tag="bc", bufs=8, name="f_b")
                    nc.vector.tensor_mul(f_b, dot_b, rvnsb[j])
                    m2 = st.tile([128, TC], F32, tag="m2", bufs=2, name="m2")
                    nc.vector.tensor_mul(m2, vhat[j], f_b)
                    nc.vector.tensor_sub(m2, pvs, m2)
                    nc.vector.tensor_mul(aout[h][r], m2, inv_b)
                    emit_fill(3)

            # ---------------- schedule ----------------
            # chunk 0 prep runs solid (nothing to interleave with)
            for _ in prep_gen(0):
                pass
            for h in range(NH_L):
                nc.sync.dma_start(out=wo_sb[:, h * D:(h + 1) * D],
                                  in_=woL[:, h * D:(h + 1) * D])
            for j in range(NTC):
                if j + 1 < NTC:
                    fill_q.append(prep_gen(j + 1))
                if j >= 1:
                    fill_q.append(outproj_gen(j - 1))
                attention(j)
                drain_fill()
            for _ in outproj_gen(NTC - 1):
                pass

    nc.compile()
    return nc


def _host_inputs(x, cos, sin, w_norm, wq, wk, wv, wo):
    """Build the 8 per-core input maps (host-side layout prep only)."""
    bf = ml_dtypes.bfloat16
    wn = w_norm.astype(np.float32)
    cosT = cos.T.astype(np.float32)                                # [64, T]
    sinT = sin.T.astype(np.float32)
    sinS = np.concatenate([-sinT[:32], sinT[32:]], axis=0)         # [64, T]
    cs = np.ascontiguousarray(
        np.concatenate([cosT, sinS], axis=0), dtype=np.float32)    # [128, T]
    xb = x.astype(bf)
    xTts = [np.ascontiguousarray(xb[b].reshape(T, DC, 128).transpose(2, 1, 0))
            for b in range(B)]
    xRs = [np.ascontiguousarray(xb[b]) for b in range(B)]
    in_maps = []
    for c in range(8):
        b, g = divmod(c, 4)
        wq_s = wq[g * NH_L * HD:(g + 1) * NH_L * HD] * wn[None, :]
        wk_s = wk[g * HD:(g + 1) * HD] * wn[None, :]
        wv_s = wv[g * HD:(g + 1) * HD] * wn[None, :]
        wqkvT = np.ascontiguousarray(
            np.concatenate([wq_s, wk_s, wv_s], axis=0).T).astype(bf)  # [D, 768]
        # woL[p, h*D + d] = wo[d, g*512 + h*128 + p]
        woL = np.ascontiguousarray(
            wo[:, g * NH_L * HD:(g + 1) * NH_L * HD]                 # [D, 512]
            .T.reshape(NH_L, HD, D).transpose(1, 0, 2).reshape(HD, NH_L * D)
        ).astype(bf)                                                 # [128, 4*D]
        in_maps.append({
            "xTt": xTts[b],
            "xR": xRs[b],
            "wqkvT": wqkvT,
            "woL": woL,
            "cs": cs,
        })
    return in_maps


def kernel(x, cos, sin, w_norm, wq, wk, wv, wo, rope_dims=64, use_xsa=1,
           **_unused):
    if "nc" not in _CACHE:
        _CACHE["nc"] = _build_nc()
    nc = _CACHE["nc"]
    in_maps = _host_inputs(
        np.asarray(x), np.asarray(cos), np.asarray(sin), np.asarray(w_norm),
        np.asarray(wq), np.asarray(wk), np.asarray(wv), np.asarray(wo),
    )
    res_obj = run_bass_kernel_spmd(nc, in_maps, list(range(8)))
    _CACHE["last"] = res_obj
    res = res_obj.results
    out = np.zeros((B, T, D), dtype=np.float32)
    for c in range(8):
        b = c // 4
        out[b] += np.asarray(res[c]["out"], dtype=np.float32)
    return out
